# revision 2
# baseline (speedup 1.0000x reference)
"""GATv2Conv (H=4, D=32) on 8 Trainium2 NeuronCores — v3: tuned baseline.

Same architecture as the baseline (node-major blocks, SWDGE row gathers,
no cross-range permutation), plus:
  * bf16 feature tables and phase-B tiles: halves gather HBM traffic and
    SBUF footprint, enables DVE 2x modes on the big elementwise passes
  * phase-A output DMAs batched 4x via one 3D-AP DMA per 512-row chunk
  * idx/mask preloaded in one DMA each (bf16 mask)
  * output rows staged and written every 4 rounds
"""

import os
from contextlib import ExitStack

import numpy as np
import ml_dtypes

P = 128
H = 4
D = 32
HD = 128  # 128
FIN = 128
CH = 512  # phase-A chunk of node rows


# --------------------------------------------------------------------------
# host-side graph plan (identical to baseline)
# --------------------------------------------------------------------------
def build_plan(src, dst, n_nodes, n_cores):
    s_all = np.concatenate([src.astype(np.int64), np.arange(n_nodes, dtype=np.int64)])
    d_all = np.concatenate([dst.astype(np.int64), np.arange(n_nodes, dtype=np.int64)])
    deg = np.bincount(d_all, minlength=n_nodes)
    perm = np.argsort(-deg, kind="stable")  # position -> node, degree desc
    pos = np.empty(n_nodes, np.int64)
    pos[perm] = np.arange(n_nodes)

    nb = -(-n_nodes // P)  # real 128-node blocks
    rounds = -(-nb // n_cores)
    nb_pad = rounds * n_cores
    npos_pad = nb_pad * P

    lbar = np.maximum(
        np.where(np.arange(rounds) * n_cores * P < n_nodes,
                 deg[perm[np.minimum(np.arange(rounds) * n_cores * P,
                                     n_nodes - 1)]], 1),
        1).astype(np.int64)
    col_off = np.concatenate([[0], np.cumsum(lbar)])
    tot_l = int(col_off[-1])

    blocks = np.arange(nb_pad)
    r_of = blocks // n_cores
    j = blocks % n_cores
    core_of = np.where(r_of % 2 == 0, j, n_cores - 1 - j)

    epos = pos[d_all]
    order = np.argsort(epos, kind="stable")
    eps = epos[order]
    starts = np.zeros(n_nodes + 1, np.int64)
    np.cumsum(deg[perm], out=starts[1:])
    k = np.arange(eps.size) - starts[eps]
    blk = eps // P
    slot = eps % P
    rr = blk // n_cores
    col = col_off[rr] + k
    core_e = core_of[blk]
    sval = s_all[order]

    idx_arr = np.zeros((n_cores, P, tot_l), np.int32)
    mask_arr = np.zeros((n_cores, P, tot_l), np.float32)
    idx_arr[core_e, slot, col] = sval.astype(np.int32)
    mask_arr[core_e, slot, col] = 1.0

    if npos_pad > n_nodes:
        dpos = np.arange(n_nodes, npos_pad)
        dblk = dpos // P
        mask_arr[core_of[dblk], dpos % P, col_off[dblk // n_cores]] = 1.0

    q = np.arange(npos_pad)
    qblk = q // P
    posgrid = np.full((n_cores, rounds * P), -1, np.int64)
    posgrid[core_of[qblk], (qblk // n_cores) * P + q % P] = np.where(
        q < n_nodes, perm[np.minimum(q, n_nodes - 1)], -1)

    return dict(
        rounds=rounds, lbar=lbar.tolist(), col_off=col_off.tolist(),
        tot_l=tot_l, idx_arr=idx_arr, mask_arr=mask_arr, posgrid=posgrid,
        shard_rows=rounds * P,
    )


# --------------------------------------------------------------------------
# device program (one SPMD NEFF for all cores)
# --------------------------------------------------------------------------
def build_device_program(n_table_pad, shard_rows, rounds, lbar, col_off, tot_l):
    import concourse.bass as bass
    import concourse.tile as tile
    from concourse import bacc, mybir
    from concourse.masks import make_identity

    FP32 = mybir.dt.float32
    BF16 = mybir.dt.bfloat16
    I32 = mybir.dt.int32
    A = mybir.AluOpType
    AF = mybir.ActivationFunctionType

    nc = bacc.Bacc("TRN2", target_bir_lowering=False, debug=False)
    io = {
        "xT": nc.dram_tensor("xT", [P, n_table_pad], BF16, kind="ExternalInput").ap(),
        "xpT": nc.dram_tensor("xpT", [P, shard_rows], BF16, kind="ExternalInput").ap(),
        "W_src": nc.dram_tensor("W_src", [FIN, HD], BF16, kind="ExternalInput").ap(),
        "W_dst": nc.dram_tensor("W_dst", [FIN, HD], BF16, kind="ExternalInput").ap(),
        "b_src": nc.dram_tensor("b_src", [HD, 1], FP32, kind="ExternalInput").ap(),
        "b_dst": nc.dram_tensor("b_dst", [HD, 1], FP32, kind="ExternalInput").ap(),
        "attn_rep": nc.dram_tensor("attn_rep", [P, HD], BF16, kind="ExternalInput").ap(),
        "idx": nc.dram_tensor("idx", [P, tot_l], I32, kind="ExternalInput").ap(),
        "mask": nc.dram_tensor("mask", [P, tot_l], BF16, kind="ExternalInput").ap(),
        "out": nc.dram_tensor("out", [shard_rows, HD], FP32, kind="ExternalOutput").ap(),
    }
    fs_table = nc.dram_tensor("fs_table", [n_table_pad, HD], BF16, kind="Internal").ap()
    fd_shard = nc.dram_tensor("fd_shard", [shard_rows, HD], BF16, kind="Internal").ap()

    with tile.TileContext(nc) as tc:
        with ExitStack() as ctx:
            consts = ctx.enter_context(tc.tile_pool(name="consts", bufs=1))
            wsrc_t = consts.tile([FIN, HD], BF16)
            nc.sync.dma_start(wsrc_t[:], io["W_src"][:, :])
            wdst_t = consts.tile([FIN, HD], BF16)
            nc.sync.dma_start(wdst_t[:], io["W_dst"][:, :])
            bsrc_t = consts.tile([HD, 1], FP32)
            nc.sync.dma_start(bsrc_t[:], io["b_src"][:, :])
            bdst_t = consts.tile([HD, 1], FP32)
            nc.sync.dma_start(bdst_t[:], io["b_dst"][:, :])
            attn_t = consts.tile([P, HD], BF16)
            nc.sync.dma_start(attn_t[:], io["attn_rep"][:, :])
            ident = consts.tile([P, P], FP32)
            make_identity(nc, ident[:])
            idx_all = consts.tile([P, tot_l], I32)
            nc.sync.dma_start(idx_all[:], io["idx"][:, :])
            mask_all = consts.tile([P, tot_l], BF16)
            nc.sync.dma_start(mask_all[:], io["mask"][:, :])

            # ---------------- phase A: feature tables --------------------
            with ExitStack() as actx:
                apool = actx.enter_context(tc.tile_pool(name="pha", bufs=3))
                apsum = actx.enter_context(tc.tile_pool(name="phamm", bufs=2, space="PSUM"))
                tpsum = actx.enter_context(tc.tile_pool(name="phatr", bufs=4, space="PSUM"))

                def phase_a(src_ap, w_t, b_t, dst_table, nrows):
                    for c0 in range(0, nrows, CH):
                        cw = min(CH, nrows - c0)
                        xt = apool.tile([P, CH], BF16, tag="xt")
                        nc.sync.dma_start(xt[:, :cw], src_ap[:, c0:c0 + cw])
                        ps = apsum.tile([P, CH], FP32, tag="mm")
                        nc.tensor.matmul(ps[:, :cw], lhsT=w_t[:], rhs=xt[:, :cw],
                                         start=True, stop=True)
                        fT = apool.tile([P, CH], FP32, tag="fT")
                        nc.vector.tensor_scalar(out=fT[:, :cw], in0=ps[:, :cw],
                                                scalar1=b_t[:, :], scalar2=None,
                                                op0=A.add)
                        stg = apool.tile([P, CH], BF16, tag="stg")
                        for s0 in range(0, cw, P):
                            sw = min(P, cw - s0)
                            pt = tpsum.tile([P, P], FP32, tag="tr")
                            nc.tensor.transpose(out=pt[:sw, :], in_=fT[:, s0:s0 + sw],
                                                identity=ident[:])
                            nc.scalar.copy(out=stg[:sw, s0:s0 + P], in_=pt[:sw, :])
                        # one 3D-AP DMA writes all 4 transposed 128-blocks
                        nt = cw // P
                        nc.sync.dma_start(
                            dst_table[c0:c0 + nt * P, :].rearrange(
                                "(t p) f -> p t f", p=P),
                            stg[:, :nt * P].rearrange(
                                "p (t f) -> p t f", t=nt))

                phase_a(io["xT"], wsrc_t, bsrc_t, fs_table, n_table_pad)
                phase_a(io["xpT"], wdst_t, bdst_t, fd_shard, shard_rows)

            # ---------------- phase B: per-block message passing ---------
            bpool = ctx.enter_context(tc.tile_pool(name="phb", bufs=2))
            opool = ctx.enter_context(tc.tile_pool(name="pho", bufs=2))
            OB = 4  # rounds per output batch
            ot = None
            for r in range(rounds):
                L = int(lbar[r])
                off = int(col_off[r])
                LH = L * H
                LHD = L * HD

                fd_t = bpool.tile([P, HD], BF16, tag="fd")
                nc.sync.dma_start(fd_t[:], fd_shard[r * P:(r + 1) * P, :])

                fs_g = bpool.tile([P, LHD], BF16, tag="fsg")
                for l in range(L):
                    nc.gpsimd.indirect_dma_start(
                        out=fs_g[:, l * HD:(l + 1) * HD], out_offset=None,
                        in_=fs_table[:, :],
                        in_offset=bass.IndirectOffsetOnAxis(
                            ap=idx_all[:, off + l:off + l + 1], axis=0))

                # t = fs_g + broadcast(fd)
                t = bpool.tile([P, LHD], BF16, tag="t")
                nc.vector.tensor_tensor(
                    out=t[:].rearrange("p (l f) -> p l f", l=L),
                    in0=fs_g[:].rearrange("p (l f) -> p l f", l=L),
                    in1=fd_t[:, None, :].to_broadcast([P, L, HD]), op=A.add)

                # u = LeakyReLU(t) = max(0.2t, t)
                u = bpool.tile([P, LHD], BF16, tag="u")
                nc.vector.scalar_tensor_tensor(out=u[:], in0=t[:], scalar=0.2,
                                               in1=t[:], op0=A.mult, op1=A.max)

                # v = u * attn ; scr = sum_d v
                v = bpool.tile([P, LHD], BF16, tag="t")
                nc.vector.tensor_tensor(
                    out=v[:].rearrange("p (l f) -> p l f", l=L),
                    in0=u[:].rearrange("p (l f) -> p l f", l=L),
                    in1=attn_t[:, None, :].to_broadcast([P, L, HD]), op=A.mult)
                scr = bpool.tile([P, LH], BF16, tag="scr")
                with nc.allow_low_precision(reason="bf16 scores ok at 2e-2 tol"):
                    nc.vector.tensor_reduce(
                        out=scr[:].rearrange("p (l h) -> p l h", h=H),
                        in_=v[:].rearrange("p (l h d) -> p l h d", h=H, d=D),
                        axis=mybir.AxisListType.X, op=A.add)

                # es = exp(scr) * mask
                es0 = bpool.tile([P, LH], FP32, tag="es0")
                nc.scalar.activation(out=es0[:], in_=scr[:], func=AF.Exp)
                es = bpool.tile([P, LH], BF16, tag="es")
                nc.vector.tensor_tensor(
                    out=es[:].rearrange("p (l h) -> p l h", h=H),
                    in0=es0[:].rearrange("p (l h) -> p l h", h=H),
                    in1=mask_all[:, off:off + L, None].to_broadcast([P, L, H]),
                    op=A.mult)

                # den = sum_l es ; rden = 1/den
                den = bpool.tile([P, H], FP32, tag="den")
                nc.vector.tensor_reduce(
                    out=den[:], in_=es[:].rearrange("p (l h) -> p h l", h=H),
                    axis=mybir.AxisListType.X, op=A.add)
                rden = bpool.tile([P, H], FP32, tag="rden")
                nc.vector.reciprocal(out=rden[:], in_=den[:])

                # w = es * fs_g ; agg = sum_l w
                w = bpool.tile([P, LHD], BF16, tag="u")
                nc.vector.tensor_tensor(
                    out=w[:].rearrange("p (l h d) -> p l h d", h=H, d=D),
                    in0=fs_g[:].rearrange("p (l h d) -> p l h d", h=H, d=D),
                    in1=es[:].rearrange("p (l h) -> p l h", h=H)[:, :, :, None]
                        .to_broadcast([P, L, H, D]),
                    op=A.mult)
                agg = bpool.tile([P, HD], FP32, tag="agg")
                nc.vector.tensor_reduce(
                    out=agg[:], in_=w[:].rearrange("p (l f) -> p f l", l=L),
                    axis=mybir.AxisListType.X, op=A.add)

                # out = relu(agg * rden)
                rden_rep = rden[:, :, None].to_broadcast([P, H, D])
                if r % OB == 0:
                    ot = opool.tile([P, OB * HD], FP32, tag="ot")
                sc = bpool.tile([P, HD], FP32, tag="sc")
                nc.vector.tensor_tensor(
                    out=sc[:].rearrange("p (h d) -> p h d", h=H),
                    in0=agg[:].rearrange("p (h d) -> p h d", h=H),
                    in1=rden_rep, op=A.mult)
                ob = r % OB
                nc.vector.tensor_scalar(out=ot[:, ob * HD:(ob + 1) * HD],
                                        in0=sc[:], scalar1=0.0,
                                        scalar2=None, op0=A.max)
                if r % OB == OB - 1 or r == rounds - 1:
                    r0 = (r // OB) * OB
                    nt = r - r0 + 1
                    nc.sync.dma_start(
                        io["out"][r0 * P:(r0 + nt) * P, :].rearrange(
                            "(t p) f -> p t f", p=P),
                        ot[:, :nt * HD].rearrange("p (t f) -> p t f", t=nt))

    nc.compile()
    return nc, io


# --------------------------------------------------------------------------
# full kernel: plan -> build -> run on 8 cores -> assemble
# --------------------------------------------------------------------------
_NC_CACHE = {}
_PLAN_CACHE = {}


def kernel(x, src, dst, W_src, b_src, W_dst, b_dst, attn, _trace=False):
    import hashlib
    n_cores = 8
    n = x.shape[0]
    src = np.asarray(src)
    dst = np.asarray(dst)
    pkey = hashlib.sha1(src.tobytes() + dst.tobytes()).hexdigest()
    plan = _PLAN_CACHE.get(pkey)
    if plan is None:
        plan = build_plan(src, dst, n, n_cores)
        _PLAN_CACHE[pkey] = plan
    rounds, shard_rows, tot_l = plan["rounds"], plan["shard_rows"], plan["tot_l"]
    n_table_pad = -(-n // CH) * CH

    key = (n_table_pad, shard_rows, rounds, tuple(plan["lbar"]))
    if key in _NC_CACHE:
        nc, io = _NC_CACHE[key]
    else:
        nc, io = build_device_program(
            n_table_pad, shard_rows, rounds, plan["lbar"], plan["col_off"], tot_l)
        _NC_CACHE[key] = (nc, io)

    x = np.asarray(x, np.float32)
    xT = np.zeros((P, n_table_pad), ml_dtypes.bfloat16)
    xT[:, :n] = x.T.astype(ml_dtypes.bfloat16)
    attn_rep = np.broadcast_to(np.asarray(attn, np.float32).reshape(1, HD),
                               (P, HD)).astype(ml_dtypes.bfloat16).copy()
    w_src = np.asarray(W_src, np.float32).astype(ml_dtypes.bfloat16)
    w_dst = np.asarray(W_dst, np.float32).astype(ml_dtypes.bfloat16)
    b_src_c = np.asarray(b_src, np.float32).reshape(HD, 1).copy()
    b_dst_c = np.asarray(b_dst, np.float32).reshape(HD, 1).copy()

    in_maps = []
    for c in range(n_cores):
        nodes = plan["posgrid"][c]
        xp = np.zeros((shard_rows, FIN), np.float32)
        valid = nodes >= 0
        xp[valid] = x[nodes[valid]]
        in_maps.append({
            "xT": xT,
            "xpT": np.ascontiguousarray(xp.T).astype(ml_dtypes.bfloat16),
            "W_src": w_src, "W_dst": w_dst,
            "b_src": b_src_c, "b_dst": b_dst_c,
            "attn_rep": attn_rep,
            "idx": np.ascontiguousarray(plan["idx_arr"][c]),
            "mask": np.ascontiguousarray(
                plan["mask_arr"][c]).astype(ml_dtypes.bfloat16),
        })

    from concourse.bass_utils import run_bass_kernel_spmd
    res = run_bass_kernel_spmd(nc, in_maps, core_ids=list(range(n_cores)),
                               trace=_trace, stitch_traces=_trace,
                               trace_cores=list(range(n_cores)) if _trace else None)

    out_full = np.zeros((n, HD), np.float32)
    for c in range(n_cores):
        nodes = plan["posgrid"][c]
        valid = nodes >= 0
        out_full[nodes[valid]] = res.results[c]["out"][valid]
    if _trace:
        return out_full, res
    return out_full
